# revision 11
# baseline (speedup 1.0000x reference)
"""MinCut refinement loss on 8 Trainium2 NeuronCores.

Math: with per-edge RBF weights w_e = exp(-||x_src - x_tgt||^2 / 2) and
segment softmax S = softmax(x @ W + b),
    assoc_k = degree @ S[:, k]             == sum_e w_e * S[src_e, k]
    cut_k   = sum_e w_e * S[src_e,k] * (1 - S[tgt_e,k])
            == assoc_k - sum_e w_e * S[src_e,k] * S[tgt_e,k]
so no N-length scatter is needed: everything is a per-edge gather reduced
into K=16 accumulators.

Device strategy (edges sharded 8 ways):
  Phase A (per core): compute S (fp32, also the graded S output) and write a
  fused node table  row_i = [x_i as bf16 (256B) | S_i as bf16 (32B) | pad]
  = 512B rows, so ONE dma_gather descriptor per edge endpoint fetches both
  features and softmax values at full DMA line rate.
  Phase B: per tile of NI edges, gather src/tgt rows, compute
  w = exp(-0.5 * sum((xs-xt)^2)) on DVE/ACT, accumulate w*Ss and w*Ss*St
  into [128, G, 16] fp32 accumulators.
  Host: sums the per-core accumulators and forms the loss (the unshard step).

dma_gather indices are int16, so edges are bucketed by (src<32768, tgt<32768)
and hi-half gathers use a sliced table base. Pad slots use idx 0 with a
per-tile weight mask of 0 so they contribute exactly nothing.
"""

import sys

sys.path.insert(0, "/opt/trn_rl_repo")

import numpy as np

N_NODES = 50000
D_FEAT = 128
K_SEG = 16
N_CORES = 8
HALF = 32768
NI = 4096            # edges per gather instruction
G = NI // 128        # 32 column-groups in gathered tiles
ROW = 256            # table row: 256 bf16 = 512 bytes
EPS = 1e-8

_COMPILED = {}


def _build_program(tiles_per_bucket):
    """Build the SPMD Bass program. tiles_per_bucket: list of 4 ints, tiles
    for buckets (src_hi, tgt_hi) in order (0,0),(0,1),(1,0),(1,1)."""
    import concourse.bacc as bacc
    import concourse.mybir as mybir
    from concourse.tile import TileContext

    T = sum(tiles_per_bucket)
    f32 = mybir.dt.float32
    bf16 = mybir.dt.bfloat16
    i16 = mybir.dt.int16

    nc = bacc.Bacc("TRN2", target_bir_lowering=False, debug=False,
                   num_devices=N_CORES)

    x_in = nc.dram_tensor("x_in", [N_NODES, D_FEAT], f32, kind="ExternalInput")
    id_in = nc.dram_tensor("id_in", [128, 128], f32, kind="ExternalInput")
    w_in = nc.dram_tensor("w_in", [D_FEAT, K_SEG], f32, kind="ExternalInput")
    b_in = nc.dram_tensor("b_in", [1, K_SEG], f32, kind="ExternalInput")
    sidx_in = nc.dram_tensor("sidx_in", [T, 128, NI // 16], i16, kind="ExternalInput")
    tidx_in = nc.dram_tensor("tidx_in", [T, 128, NI // 16], i16, kind="ExternalInput")
    mask_in = nc.dram_tensor("mask_in", [T, 128, G], f32, kind="ExternalInput")

    table = nc.dram_tensor("table", [N_NODES, ROW], bf16, kind="Internal")
    s_out = nc.dram_tensor("s_out", [N_NODES, K_SEG], f32, kind="ExternalOutput")
    acc_out = nc.dram_tensor("acc_out", [2, 128, G * K_SEG], f32, kind="ExternalOutput")

    # phase A groups: up to 8 node-subtiles (of <=128 rows) per group
    groups = []
    r = 0
    while r < N_NODES:
        rows = min(1024, N_NODES - r)
        subs = []
        rr = r
        while rr < r + rows:
            subs.append(min(128, r + rows - rr))
            rr += subs[-1]
        groups.append((r, subs))
        r += rows

    with TileContext(nc) as tc:
        with (
            tc.tile_pool(name="const", bufs=1) as cpool,
            tc.tile_pool(name="pa", bufs=2) as pa,
            tc.tile_pool(name="pa_ps", bufs=2, space="PSUM") as pa_ps,
            tc.tile_pool(name="pb_big", bufs=2) as pb_big,
            tc.tile_pool(name="pb", bufs=2) as pb,
            tc.tile_pool(name="acc", bufs=1) as accp,
        ):
            w_sb = cpool.tile([128, K_SEG], f32)
            nc.sync.dma_start(w_sb[:], w_in[:, :])
            b_sb = cpool.tile([1, K_SEG], f32)
            nc.sync.dma_start(b_sb[:], b_in[:, :])
            ones_sb = cpool.tile([1, 128], f32)
            nc.vector.memset(ones_sb[:], 1.0)
            ident = cpool.tile([128, 128], f32)
            nc.sync.dma_start(ident[:], id_in[:, :])

            acc_a = accp.tile([128, G, K_SEG], f32)
            nc.vector.memset(acc_a[:], 0.0)
            acc_c = accp.tile([128, G, K_SEG], f32)
            nc.vector.memset(acc_c[:], 0.0)

            # ---------------- Phase A: S + fused table ----------------
            for (r0, subs) in groups:
                ns = len(subs)
                partial = any(s < 128 for s in subs)
                xt = pa.tile([128, ns, D_FEAT], f32, tag="xt")
                xT_ps = pa_ps.tile([128, ns, 128], f32, tag="xT_ps")
                xT_sb = pa.tile([128, ns, 128], f32, tag="xT_sb")
                lg_ps = pa_ps.tile([128, ns, K_SEG], f32, tag="lg_ps")
                expl = pa.tile([128, ns, K_SEG], f32, tag="expl")
                ssum = pa.tile([128, ns], f32, tag="ssum")
                rcp = pa.tile([128, ns], f32, tag="rcp")
                s_sb = pa.tile([128, ns, K_SEG], f32, tag="s_sb")
                row_sb = pa.tile([128, ns, ROW], bf16, tag="row_sb")

                nc.vector.memset(row_sb[:], 0.0)
                if not partial:
                    nc.sync.dma_start(
                        xt[:],
                        x_in[r0:r0 + 128 * ns, :].rearrange(
                            "(t p) f -> p t f", p=128))
                else:
                    rr = r0
                    for s, rows in enumerate(subs):
                        nc.sync.dma_start(xt[:rows, s, :], x_in[rr:rr + rows, :])
                        rr += rows
                rr = r0
                for s, rows in enumerate(subs):
                    # xT = x^T  (features on partitions)
                    nc.tensor.transpose(xT_ps[:, s, :rows], xt[:rows, s, :],
                                        ident[:rows, :rows])
                    nc.vector.tensor_copy(xT_sb[:, s, :rows], xT_ps[:, s, :rows])
                    # logits = x @ W + b  (nodes on partitions)
                    nc.tensor.matmul(lg_ps[:rows, s, :], xT_sb[:, s, :rows],
                                     w_sb[:, :], start=True, stop=False)
                    nc.tensor.matmul(lg_ps[:rows, s, :], ones_sb[:, :rows],
                                     b_sb[:, :], start=False, stop=True)
                    rr += rows
                # softmax over K (free dim); logits are O(1), skip max-sub
                if not partial:
                    nc.scalar.activation(expl[:], lg_ps[:],
                                         mybir.ActivationFunctionType.Exp)
                    nc.vector.reduce_sum(ssum[:], expl[:],
                                         axis=mybir.AxisListType.X)
                    nc.vector.reciprocal(rcp[:], ssum[:])
                    nc.vector.tensor_mul(s_sb[:], expl[:],
                                         rcp[:].to_broadcast([128, ns, K_SEG]))
                    nc.vector.tensor_copy(row_sb[:, :, 0:D_FEAT], xt[:, :, :])
                    nc.vector.tensor_copy(row_sb[:, :, D_FEAT:D_FEAT + K_SEG],
                                          s_sb[:])
                else:
                    for s, rows in enumerate(subs):
                        nc.scalar.activation(expl[:rows, s, :], lg_ps[:rows, s, :],
                                             mybir.ActivationFunctionType.Exp)
                        nc.vector.reduce_sum(ssum[:rows, s:s + 1],
                                             expl[:rows, s, :],
                                             axis=mybir.AxisListType.X)
                        nc.vector.reciprocal(rcp[:rows, s:s + 1],
                                             ssum[:rows, s:s + 1])
                        nc.vector.tensor_mul(
                            s_sb[:rows, s, :], expl[:rows, s, :],
                            rcp[:rows, s:s + 1].to_broadcast([rows, K_SEG]))
                        nc.vector.tensor_copy(row_sb[:rows, s, 0:D_FEAT],
                                              xt[:rows, s, :])
                        nc.vector.tensor_copy(
                            row_sb[:rows, s, D_FEAT:D_FEAT + K_SEG],
                            s_sb[:rows, s, :])
                if not partial:
                    nc.sync.dma_start(
                        s_out[r0:r0 + 128 * ns, :].rearrange(
                            "(t p) k -> p t k", p=128), s_sb[:])
                    nc.sync.dma_start(
                        table[r0:r0 + 128 * ns, :].rearrange(
                            "(t p) e -> p t e", p=128), row_sb[:])
                else:
                    rr = r0
                    for s, rows in enumerate(subs):
                        nc.sync.dma_start(s_out[rr:rr + rows, :],
                                          s_sb[:rows, s, :])
                        nc.sync.dma_start(table[rr:rr + rows, :],
                                          row_sb[:rows, s, :])
                        rr += rows

            # ---------------- Phase B: edge tiles ----------------
            bucket_of_tile = []
            for bidx, tb in enumerate(tiles_per_bucket):
                bucket_of_tile += [bidx] * tb
            for t in range(T):
                bidx = bucket_of_tile[t]
                s_hi, t_hi = bidx // 2, bidx % 2
                si = pb.tile([128, NI // 16], i16, tag="si")
                nc.sync.dma_start(si[:], sidx_in[t, :, :])
                ti = pb.tile([128, NI // 16], i16, tag="ti")
                nc.sync.dma_start(ti[:], tidx_in[t, :, :])
                mk = pb.tile([128, G], f32, tag="mk")
                nc.sync.dma_start(mk[:], mask_in[t, :, :])

                gs = pb_big.tile([128, G, ROW], bf16, tag="gs")
                src_base = table[HALF:, :] if s_hi else table[:, :]
                nc.gpsimd.dma_gather(gs[:], src_base, si[:], NI, NI, ROW,
                                     single_packet=False)
                gt = pb_big.tile([128, G, ROW], bf16, tag="gt")
                tgt_base = table[HALF:, :] if t_hi else table[:, :]
                nc.gpsimd.dma_gather(gt[:], tgt_base, ti[:], NI, NI, ROW,
                                     single_packet=False)

                d = pb_big.tile([128, G, D_FEAT], bf16, tag="d")
                nc.vector.tensor_sub(d[:], gs[:, :, 0:D_FEAT], gt[:, :, 0:D_FEAT])
                nc.vector.tensor_mul(d[:], d[:], d[:])
                dist = pb.tile([128, G], f32, tag="dist")
                nc.vector.reduce_sum(dist[:], d[:], axis=mybir.AxisListType.X)
                w_t = pb.tile([128, G], f32, tag="w_t")
                nc.scalar.activation(w_t[:], dist[:],
                                     mybir.ActivationFunctionType.Exp, scale=-0.5)
                nc.vector.tensor_mul(w_t[:], w_t[:], mk[:])

                wss = pb.tile([128, G, K_SEG], bf16, tag="wss")
                nc.vector.tensor_mul(wss[:], gs[:, :, D_FEAT:D_FEAT + K_SEG],
                                     w_t[:].to_broadcast([128, G, K_SEG]))
                nc.vector.tensor_add(acc_a[:], acc_a[:], wss[:])
                crs = pb.tile([128, G, K_SEG], bf16, tag="crs")
                nc.vector.tensor_mul(crs[:], wss[:],
                                     gt[:, :, D_FEAT:D_FEAT + K_SEG])
                nc.vector.tensor_add(acc_c[:], acc_c[:], crs[:])

            nc.sync.dma_start(acc_out[0, :, :],
                              acc_a[:].rearrange("p g k -> p (g k)"))
            nc.sync.dma_start(acc_out[1, :, :],
                              acc_c[:].rearrange("p g k -> p (g k)"))
    nc.compile()
    return nc


def _wrap_idx(vals):
    """(T, NI) int16 -> (T, 128, NI//16) in the HW layout: index j of tile t
    at partition 16*q + (j%16), column j//16, replicated for all 8 groups q."""
    Tn = vals.shape[0]
    base = vals.reshape(Tn, NI // 16, 16).transpose(0, 2, 1)  # (T, 16, NI/16)
    return np.tile(base, (1, 8, 1)).astype(np.int16)


def _shard(edge_index):
    """Bucket + pad each core's edge shard. Returns (tiles_per_bucket,
    per-core dicts of sidx/tidx/mask arrays)."""
    E = edge_index.shape[1]
    Ec = E // N_CORES
    assert Ec * N_CORES == E
    src_all = np.asarray(edge_index[0], dtype=np.int64)
    tgt_all = np.asarray(edge_index[1], dtype=np.int64)

    per_core = []
    counts = np.zeros((N_CORES, 4), np.int64)
    for c in range(N_CORES):
        s = src_all[c * Ec:(c + 1) * Ec]
        t = tgt_all[c * Ec:(c + 1) * Ec]
        b = (s >= HALF).astype(np.int8) * 2 + (t >= HALF).astype(np.int8)
        order = np.argsort(b, kind="stable")
        s, t, b = s[order], t[order], b[order]
        edges_by_b = []
        for bidx in range(4):
            m = b == bidx
            counts[c, bidx] = int(m.sum())
            edges_by_b.append((s[m], t[m]))
        per_core.append(edges_by_b)

    tiles_per_bucket = [int(-(-counts[:, bidx].max() // NI)) for bidx in range(4)]
    T = sum(tiles_per_bucket)

    core_inputs = []
    for c in range(N_CORES):
        sidx = np.zeros((T, NI), np.int16)
        tidx = np.zeros((T, NI), np.int16)
        mask = np.zeros((T, NI), np.float32)
        t0 = 0
        for bidx in range(4):
            s, t = per_core[c][bidx]
            n = s.shape[0]
            s_loc = (s - (HALF if bidx // 2 else 0)).astype(np.int16)
            t_loc = (t - (HALF if bidx % 2 else 0)).astype(np.int16)
            flat_s = sidx[t0:t0 + tiles_per_bucket[bidx]].reshape(-1)
            flat_t = tidx[t0:t0 + tiles_per_bucket[bidx]].reshape(-1)
            flat_m = mask[t0:t0 + tiles_per_bucket[bidx]].reshape(-1)
            flat_s[:n] = s_loc
            flat_t[:n] = t_loc
            flat_m[:n] = 1.0
            t0 += tiles_per_bucket[bidx]
        # mask layout must match gather OUTPUT layout: edge j -> (j%128, j//128)
        mask_pg = mask.reshape(T, G, 128).transpose(0, 2, 1).copy()
        core_inputs.append({
            "sidx_in": _wrap_idx(sidx),
            "tidx_in": _wrap_idx(tidx),
            "mask_in": np.ascontiguousarray(mask_pg),
        })
    return tiles_per_bucket, core_inputs


LAST_RESULT = None


def kernel(gat_refined_patch_features, patch_graph_edge_index,
           num_expected_segments, W_seg, b_seg, _profile=False):
    global LAST_RESULT
    from concourse.bass_utils import run_bass_kernel_spmd

    x = np.ascontiguousarray(np.asarray(gat_refined_patch_features, np.float32))
    ei = np.asarray(patch_graph_edge_index)
    W = np.ascontiguousarray(np.asarray(W_seg, np.float32))
    b = np.ascontiguousarray(np.asarray(b_seg, np.float32).reshape(1, K_SEG))

    tiles_per_bucket, core_inputs = _shard(ei)
    key = tuple(tiles_per_bucket)
    if key not in _COMPILED:
        _COMPILED[key] = _build_program(tiles_per_bucket)
    nc = _COMPILED[key]

    ident = np.eye(128, dtype=np.float32)
    in_maps = []
    for c in range(N_CORES):
        m = {"x_in": x, "w_in": W, "b_in": b, "id_in": ident}
        m.update(core_inputs[c])
        in_maps.append(m)

    res = run_bass_kernel_spmd(nc, in_maps, core_ids=list(range(N_CORES)),
                               trace=bool(_profile))
    LAST_RESULT = res

    S = np.asarray(res.results[0]["s_out"], np.float32)
    assoc = np.zeros(K_SEG, np.float64)
    cross = np.zeros(K_SEG, np.float64)
    for c in range(N_CORES):
        acc = np.asarray(res.results[c]["acc_out"], np.float64)
        assoc += acc[0].reshape(-1, K_SEG).sum(axis=0)
        cross += acc[1].reshape(-1, K_SEG).sum(axis=0)
    cut = assoc - cross
    loss = np.where(assoc > EPS, cut / np.maximum(assoc, 1e-300), 0.0).sum()
    return np.float32(loss), S
